# revision 33
# baseline (speedup 1.0000x reference)
"""Trainium2 Bass kernel for explicit multi-head attention.

Problem: x[2, 2048, 1024], Wq/Wk/Wv/Wo[1024, 1024] (+biases), NHEAD=16.
Sharding over 8 NeuronCores: data-parallel on batch (cores 0-3 -> b=0,
cores 4-7 -> b=1), tensor-parallel over heads (4 heads per core).  Each
core computes its 4 heads' attention plus the partial out-projection
(ctx_local @ Wo[rows_local]); partials are summed on the host, which is
mathematically the all-reduce the sharding hint asks for.  The v-bias and
out-bias are linear post-softmax corrections (sum of softmax weights is
1), so bv@Wo + bo is folded in on the host and never touches the device.

Device-side layout notes:
 - Everything is computed transposed: xT [D, L] streams through the PE as
   the moving operand, producing qT/kT [d_local, L] and v [L, d_local].
 - Scores are computed transposed per (head, Lk-tile, Lq-chunk):
   sT = kT_tile^T-contraction -> [Lk=128, Lq=512]; the even/odd head
   matmuls occupy PE row-halves 0-63/64-127 and co-execute.  Softmax runs
   without max-subtraction (score range ~[-4, 4], exp is safe in fp32).
 - The softmax denominator is produced by the PV matmul itself: the
   stationary v tile carries an extra all-ones column, so one PSUM row
   accumulates sum_k exp(s).  Even head lands ctx in psum partitions
   0-63 / denom at 64; odd head (stationary window shifted) lands ctx at
   64-127 / denom at 32, so ctxT packs two heads per 128-partition tile
   with no cross-partition moves.
 - Input DMA is column-chunk-major: xT streams per (k-tile, Lq-chunk)
   piece across three HWDGE queues (sync / vector / scalar), ordered so
   chunk 0 plus wq/wk/wv land ~12-15us in; the j-loop starts ~17us
   instead of waiting for the whole k-tile-major stream (~40us).  The
   whole projection drip schedule (k(c1..3), q(c1) and all pair-1
   groups) rides inside the attention j-loops.
 - PSUM budget (8 banks): scores double-buffer 2x[128,1024] (4), ctx
   accumulators 2x[128,512] (reuse one chunk apart, released by the
   prompt raw-copies), one "bcs" bank (denominator broadcast target at
   boundaries; chunk-0 qk drips and steady-state outproj tiles reuse it
   off-cycle), one "misc" bank (v-projection / steady drips).
 - Normalization off the critical path: DVE copies evacuate raw
   ctx+denoms at the PV stop, reciprocal_approx_fast (51-ULP, ~5x
   faster than exact) + rank-1 PE broadcasts + one DVE multiply per
   head produce normalized ctxT (bf16) while the next chunk runs.
 - The out-projection runs fully in bf16 (ctxT and Wo): single-pass PE
   matmuls and fast weight loads, vs 2-cycle/col fp32.
"""

import os
import sys

import numpy as np

for _p in ("/opt/trn_rl_repo", "/root/.axon_site/_ro/trn_rl_repo"):
    if os.path.isdir(_p) and _p not in sys.path:
        sys.path.append(_p)

import concourse.bass as bass
import concourse.mybir as mybir
import concourse.tile as tile
from concourse import bacc
from concourse.bass_utils import run_bass_kernel_spmd

# ---------------------------------------------------------------------------
# ACT table-set pinning: the only activation is Exp (plus Identity/Copy for
# helpers).  The stock chooser can alternate table sets between activation
# variants, costing an ACT_TABLE_LOAD (~1.3-2.6us) per switch; restrict the
# map so exactly one set serves everything and one load is emitted.
import concourse.hw_specs as _hw_specs

_orig_get_activation_tables = _hw_specs.get_activation_tables


def _pinned_activation_tables(module_arch):
    t = _orig_get_activation_tables(module_arch)
    pin = {
        mybir.ActivationFunctionType.Exp,
        mybir.ActivationFunctionType.Ln,
        mybir.ActivationFunctionType.Identity,
        mybir.ActivationFunctionType.Copy,
    }
    out = {}
    for name, fns in t.items():
        if name == "natural_log_exp_and_others":
            out[name] = set(fns)
        else:
            out[name] = set(fns) - pin
    return out


bacc.get_activation_tables = _pinned_activation_tables

B = 2
L = 2048
D_MODEL = 1024
NHEAD = 16
D_HEAD = 64
SCALE = 1.0 / float(np.sqrt(D_HEAD))
N_CORES = 8
TP = 4                      # tensor-parallel group size (heads split)
HEADS_PER_CORE = NHEAD // TP          # 4
D_LOCAL = HEADS_PER_CORE * D_HEAD     # 256
N_PAIRS = HEADS_PER_CORE // 2         # 2 head-pairs per core
KT = D_MODEL // 128                   # 8 contraction tiles for projections
LT = L // 128                         # 16 L tiles
NCH = L // 512                        # 4 Lq chunks of 512

F32 = mybir.dt.float32
BF16 = mybir.dt.bfloat16
ATT_DT = BF16   # attention operands (qT/kT/v_aug/exp-probs/ctxT/wo)
PROJ_DT = BF16  # projection inputs (x, Wq/Wk/Wv/Wo): halves the HBM load
OUT_DT = BF16   # out_p store dtype

# v_aug stationary layout (one tile per (pair, Lk-tile), [128, 193]):
#   cols 0:64    = v(even head)          -> even window cols 0:128
#   col  64      = ones (even denom -> even psum row 64)
#   col  65      = ones (odd denom; odd window col index 0 -> odd psum row 0,
#                  so the whole ctx_o bank evacuates with one base-0 copy)
#   cols 66:129  = zeros
#   cols 129:193 = v(odd head)           -> odd window cols 65:193
VAUG_W = 193


def _to_proj(x):
    import ml_dtypes

    return np.ascontiguousarray(np.asarray(x, np.float32).astype(ml_dtypes.bfloat16))


def build_kernel():
    nc = bacc.Bacc("TRN2", target_bir_lowering=False, debug=False)

    xT = nc.dram_tensor("xT", [D_MODEL, L], PROJ_DT, kind="ExternalInput").ap()
    wq = nc.dram_tensor("wq", [D_MODEL, D_LOCAL], PROJ_DT, kind="ExternalInput").ap()
    wk = nc.dram_tensor("wk", [D_MODEL, D_LOCAL], PROJ_DT, kind="ExternalInput").ap()
    wv = nc.dram_tensor("wv", [D_MODEL, D_LOCAL], PROJ_DT, kind="ExternalInput").ap()
    wo = nc.dram_tensor("wo", [D_LOCAL, D_MODEL], PROJ_DT, kind="ExternalInput").ap()
    bq = nc.dram_tensor("bq", [D_LOCAL], F32, kind="ExternalInput").ap()
    bk = nc.dram_tensor("bk", [D_LOCAL], F32, kind="ExternalInput").ap()
    out_p = nc.dram_tensor("out_p", [L, D_MODEL], OUT_DT, kind="ExternalOutput").ap()

    Exp = mybir.ActivationFunctionType.Exp

    with tile.TileContext(nc) as tc:
        with (
            tc.tile_pool(name="persist", bufs=1) as persist,
            tc.tile_pool(name="exp_pool", bufs=3) as exp_pool,
            tc.tile_pool(name="raw_pool", bufs=4) as raw_pool,
            tc.tile_pool(name="recip_pool", bufs=2) as recip_pool,
            tc.tile_pool(name="out_pool", bufs=4) as out_pool,
            tc.tile_pool(name="ps_st", bufs=2, space="PSUM") as ps_st,
            tc.tile_pool(name="ps_ctx", bufs=2, space="PSUM") as ps_ctx,
            tc.tile_pool(name="ps_bcs", bufs=1, space="PSUM") as ps_bcs,
            tc.tile_pool(name="ps_misc", bufs=1, space="PSUM") as ps_misc,
        ):
            # ---- input DMAs, column-chunk-major across three HWDGE queues.
            # Gates: xT(c0) + wq by ~14us (q/k projections), wv by ~15
            # (v tiles), later chunks stream well ahead of the j-loop's
            # consumption.  Piece = [128, 512] per (k-tile, chunk).
            xT_sb = persist.tile([128, KT, L], PROJ_DT)
            wq_sb = persist.tile([128, KT, D_LOCAL], PROJ_DT)
            wk_sb = persist.tile([128, KT, D_LOCAL], PROJ_DT)
            wv_sb = persist.tile([128, KT, D_LOCAL], PROJ_DT)
            wo_sb = persist.tile([128, N_PAIRS, D_MODEL], PROJ_DT)
            bq_sb = persist.tile([128, D_LOCAL // 128], F32)
            bk_sb = persist.tile([128, D_LOCAL // 128], F32)

            def xt_piece(eng, k, c):
                eng.dma_start(
                    xT_sb[:, k, c * 512 : (c + 1) * 512],
                    xT[k * 128 : (k + 1) * 128, c * 512 : (c + 1) * 512],
                )

            # sync queue: wk, xT-even chunk 0, wv, then xT-even chunks 1-3
            nc.sync.dma_start(wk_sb[:], wk.rearrange("(k p) n -> p k n", p=128))
            for k in range(0, KT, 2):
                xt_piece(nc.sync, k, 0)
            nc.sync.dma_start(wv_sb[:], wv.rearrange("(k p) n -> p k n", p=128))
            for c in range(1, NCH):
                for k in range(0, KT, 2):
                    xt_piece(nc.sync, k, c)
            # scalar queue: biases, xT-odd chunk 0, wq, xT-odd chunks 1-3, wo
            nc.scalar.dma_start(bq_sb[:], bq.rearrange("(m p) -> p m", p=128))
            nc.scalar.dma_start(bk_sb[:], bk.rearrange("(m p) -> p m", p=128))
            for k in range(1, KT, 2):
                xt_piece(nc.scalar, k, 0)
            nc.scalar.dma_start(wq_sb[:], wq.rearrange("(k p) n -> p k n", p=128))
            for c in range(1, NCH):
                for k in range(1, KT, 2):
                    xt_piece(nc.scalar, k, c)
            nc.scalar.dma_start(wo_sb[:], wo.rearrange("(k p) n -> p k n", p=128))

            ones128 = persist.tile([128, 512], F32)
            nc.vector.memset(ones128[:], 1.0)

            qT_sb = persist.tile([128, N_PAIRS, L], ATT_DT)
            kT_sb = persist.tile([128, N_PAIRS, L], ATT_DT)
            vaug = persist.tile([128, N_PAIRS, LT, VAUG_W], ATT_DT)
            ctxT_sb = persist.tile([128, N_PAIRS, L], ATT_DT)

            nc.vector.memset(vaug[:, :, :, 64:129], 0.0)
            nc.vector.memset(vaug[:, :, :, 64:66], 1.0)

            # ---- v projection, one Lk tile (no bias: folded on host) ----
            def emit_v(lt, pool=ps_misc, tag="misc"):
                ps = pool.tile([128, D_LOCAL], F32, tag=tag)
                for k in range(KT):
                    nc.tensor.matmul(
                        ps[:],
                        xT_sb[:, k, lt * 128 : (lt + 1) * 128],
                        wv_sb[:, k, :],
                        start=(k == 0),
                        stop=(k == KT - 1),
                    )
                for p in range(N_PAIRS):
                    nc.vector.tensor_copy(
                        vaug[:, p, lt, 0:64], ps[:, p * 128 : p * 128 + 64]
                    )
                    nc.vector.tensor_copy(
                        vaug[:, p, lt, 129:193],
                        ps[:, p * 128 + 64 : p * 128 + 128],
                    )

            # ---- one qT/kT projection group: tensor t (0=q, 1=k), pair m,
            # Lq chunk c ----
            open_qk = {}

            def emit_qk(t, m, c, pool, tag, half=None):
                # half=0 emits k 0..3 (opens the PSUM group), half=1 emits
                # k 4..7 + the bias evacuation; half=None does both.
                w_sb, b_sb, dst = ((wq_sb, bq_sb, qT_sb), (wk_sb, bk_sb, kT_sb))[t]
                if half == 1:
                    ps = open_qk.pop((t, m, c))
                else:
                    ps = pool.tile([128, 512], F32, tag=tag)
                ks = {None: range(KT), 0: range(KT // 2), 1: range(KT // 2, KT)}[half]
                for k in ks:
                    nc.tensor.matmul(
                        ps[:],
                        w_sb[:, k, m * 128 : (m + 1) * 128],
                        xT_sb[:, k, c * 512 : (c + 1) * 512],
                        start=(k == 0),
                        stop=(k == KT - 1),
                    )
                if half == 0:
                    open_qk[(t, m, c)] = ps
                    return
                nc.vector.tensor_scalar_add(
                    dst[:, m, c * 512 : (c + 1) * 512], ps[:], b_sb[:, m : m + 1]
                )

            def emit_norm_copies(p, c, ctx_e, ctx_o):
                # Evacuate raw ctx+denoms: one base-0 copy per bank (denom_e
                # rides at row 64 of ctx_e, denom_o at row 0 of ctx_o), which
                # releases both ctx PSUM banks as fast as possible.
                raw_e = raw_pool.tile([128, 512], F32, tag="raw")
                raw_o = raw_pool.tile([128, 512], F32, tag="raw")
                nc.vector.tensor_copy(raw_e[0:65, :], ctx_e[0:65, :])
                nc.vector.tensor_copy(raw_o[:, :], ctx_o[:, :])
                return {"p": p, "c": c, "raw_e": raw_e, "raw_o": raw_o}

            def emit_norm_bcast(st, half):
                # broadcast one denom row across partitions with a rank-1 PE
                # matmul (a broadcast DMA costs ~6.4us issue-to-semaphore)
                if half == 0:
                    st["bcs"] = ps_bcs.tile(
                        [128, 512], F32, tag="bcs", name="bcs"
                    )
                    nc.tensor.matmul(
                        st["bcs"][0:64, :], ones128[64:65, 0:64],
                        st["raw_e"][64:65, :], start=True, stop=True,
                    )
                else:
                    nc.tensor.matmul(
                        st["bcs"][64:128, :], ones128[0:1, 0:64],
                        st["raw_o"][0:1, :], start=True, stop=True,
                    )

            def emit_norm_mul(st):
                # reciprocal_approx_fast (51-ULP, full-partition span) + the
                # normalize multiplies, producing bf16 ctxT
                p, c, bcs = st["p"], st["c"], st["bcs"]
                sl = slice(c * 512, (c + 1) * 512)
                rt = recip_pool.tile([128, 512], F32, tag="rt", name="rt")
                nc.vector.reciprocal_approx_fast(rt[:, :], bcs[:, :])
                nc.vector.tensor_mul(
                    ctxT_sb[0:64, p, sl], st["raw_e"][0:64, :], rt[0:64, :]
                )
                nc.vector.tensor_mul(
                    ctxT_sb[64:128, p, sl], st["raw_o"][64:128, :], rt[64:128, :]
                )

            def emit_outproj_tile(c, idx, pool=ps_bcs, tag="bcs"):
                # one [128, 512] tile of out[L, D] for Lq chunk c (no bias:
                # folded on host); bf16 output halves the store DMA.
                m = 4 * c + idx // 2
                n = idx % 2
                po = pool.tile([128, 512], F32, tag=tag)
                for k in range(N_PAIRS):
                    nc.tensor.matmul(
                        po[:],
                        ctxT_sb[:, k, m * 128 : (m + 1) * 128],
                        wo_sb[:, k, n * 512 : (n + 1) * 512],
                        start=(k == 0),
                        stop=(k == N_PAIRS - 1),
                    )
                ot = out_pool.tile([128, 512], OUT_DT, tag="ot")
                nc.vector.tensor_copy(ot[:], po[:])
                nc.sync.dma_start(
                    out_p[m * 128 : (m + 1) * 128, n * 512 : (n + 1) * 512], ot[:]
                )

            # ---- emission schedule ----
            # PE warm-up: dense dummy matmuls on resident data while the
            # input DMAs land, so the HAM clock-gate ramps before the real
            # projections start.
            # No PE warm-up: the cold-clock window is ~3.4us of activity
            # (absorbed by the first projection halves), while dummy matmuls
            # burn HAM power budget that gets repaid later in 4/8-throttle
            # windows.
            # head: k(pair0, c0) then q(pair0, c0) then the first two v
            # tiles -- the minimum for attention chunk 0's j-loop to start.
            # Half-granularity emission lets each start on partial xT/weights.
            emit_qk(1, 0, 0, ps_ctx, "ctx", half=0)
            emit_qk(1, 0, 0, ps_ctx, "ctx", half=1)
            emit_qk(0, 0, 0, ps_ctx, "ctx", half=0)
            emit_qk(0, 0, 0, ps_ctx, "ctx", half=1)
            emit_v(0, ps_ctx, "ctx")
            emit_v(1, ps_ctx, "ctx")

            def emit_pv(pv):
                pv_e, pv_o, pv_p, pv_j, pv_et = pv
                nc.tensor.matmul(
                    pv_e[:], vaug[:, pv_p, pv_j, 0:128], pv_et[:, 0:512],
                    start=(pv_j == 0), stop=(pv_j == LT - 1),
                )
                nc.tensor.matmul(
                    pv_o[:], vaug[:, pv_p, pv_j, 65:193], pv_et[:, 512:1024],
                    start=(pv_j == 0), stop=(pv_j == LT - 1),
                )

            # chunk 0 drips (bcs slot, free until the first boundary):
            # j -> (group, half); kT(c') needed by j=4c'.
            chunk0_drips = {
                1: ((1, 0, 1), 0), 2: ((1, 0, 1), 1),
                4: ((1, 0, 2), 0), 5: ((1, 0, 2), 1),
                8: ((1, 0, 3), 0), 9: ((1, 0, 3), 1),
                11: ((0, 0, 1), 0), 12: ((0, 0, 1), 1),
            }
            # steady-state drips (misc slot, odd j >= 5 to keep the chunk
            # boundary region clear), chunk_i -> groups.  Chunks 5-7 carry
            # the out-projection stream and stay drip-free; chunks 3-4
            # (exp-paced, PE slack) absorb the extra pair-1 q groups.
            qk_drip = {
                1: [(1, 1, 0), (0, 0, 2)],
                2: [(1, 1, 1), (0, 0, 3)],
                3: [(1, 1, 2), (0, 1, 0), (0, 1, 3)],
                4: [(1, 1, 3), (0, 1, 1), (0, 1, 2)],
            }

            outproj_q = []
            pending = None        # chunk awaiting normalize (flush + copies)
            norm = None           # normalize stage state across j=2..4
            pending_pv = None     # software-pipelined PV emission (lags 1 j)
            for p in range(N_PAIRS):
                for c in range(NCH):
                    chunk_i = p * NCH + c
                    drips = [
                        (g, h) for g in qk_drip.get(chunk_i, []) for h in (0, 1)
                    ]
                    drip_js = iter((5, 7, 9, 11, 13, 15))
                    next_drip_j = next(drip_js) if drips else None
                    ctx_e = ps_ctx.tile([128, 512], F32, tag="ctx")
                    ctx_o = ps_ctx.tile([128, 512], F32, tag="ctx")
                    for j in range(LT):
                        sT = ps_st.tile([128, 1024], F32, tag="sT")
                        nc.tensor.matmul(
                            sT[:, 0:512],
                            kT_sb[0:64, p, j * 128 : (j + 1) * 128],
                            qT_sb[0:64, p, c * 512 : (c + 1) * 512],
                            start=True,
                            stop=True,
                        )
                        nc.tensor.matmul(
                            sT[:, 512:1024],
                            kT_sb[64:128, p, j * 128 : (j + 1) * 128],
                            qT_sb[64:128, p, c * 512 : (c + 1) * 512],
                            start=True,
                            stop=True,
                        )
                        et = exp_pool.tile([128, 1024], ATT_DT, tag="et")
                        nc.scalar.activation(et[:], sT[:], Exp, scale=SCALE)
                        if j == 0 and pending is not None:
                            # cover the last exp's latency with a filler,
                            # then flush the previous chunk's last PV
                            # (unlagged) and evacuate it -- the ACT engine
                            # streams exps seamlessly across the boundary.
                            if outproj_q:
                                emit_outproj_tile(*outproj_q.pop(0))
                            emit_pv(pending_pv)
                            pending_pv = None
                            prev_p, prev_c = pending[0], pending[1]
                            norm = emit_norm_copies(*pending)
                            pending = None
                            if prev_p == N_PAIRS - 1:
                                outproj_q.extend(
                                    (prev_c, idx) for idx in range(8)
                                )
                        elif pending_pv is not None:
                            emit_pv(pending_pv)
                        pending_pv = (ctx_e, ctx_o, p, j, et)
                        if norm is not None:
                            if j == 2:
                                emit_norm_bcast(norm, 0)
                            elif j == 3:
                                emit_norm_bcast(norm, 1)
                            elif j == 4:
                                emit_norm_mul(norm)
                                norm = None
                        if chunk_i == 0:
                            if j <= 13:
                                emit_v(j + 2)
                            if j in chunk0_drips:
                                g, h = chunk0_drips[j]
                                emit_qk(*g, ps_bcs, "bcs", half=h)
                        elif j == next_drip_j and drips:
                            g, h = drips.pop(0)
                            emit_qk(*g, ps_misc, "misc", half=h)
                            next_drip_j = next(drip_js, None)
                        if (
                            j >= 4
                            and outproj_q
                            and (j % 2 == 0 or len(outproj_q) > 4)
                        ):
                            emit_outproj_tile(*outproj_q.pop(0))
                    pending = (p, c, ctx_e, ctx_o)
            # drain the pipelined last PV, then the tail
            emit_pv(pending_pv)
            # tail: pipeline the last chunk's normalize with its
            # out-projection -- per-m-tile multiplies release each
            # out-projection tile as early as possible.
            st = emit_norm_copies(*pending)
            tp, tc_ = st["p"], st["c"]
            emit_norm_bcast(st, 0)
            emit_norm_bcast(st, 1)
            raw_e, raw_o, bcs = st["raw_e"], st["raw_o"], st["bcs"]
            outproj_q.extend((NCH - 1, idx) for idx in range(8))
            rt = recip_pool.tile([128, 512], F32, tag="rt")
            npop = 0
            for half in range(2):
                hs = slice(half * 256, (half + 1) * 256)
                nc.vector.reciprocal_approx_fast(rt[:, hs], bcs[:, hs])
                for ml in (2 * half, 2 * half + 1):
                    ms = slice(tc_ * 512 + ml * 128, tc_ * 512 + (ml + 1) * 128)
                    rs = slice(ml * 128, (ml + 1) * 128)
                    nc.vector.tensor_mul(
                        ctxT_sb[0:64, tp, ms], raw_e[0:64, rs], rt[0:64, rs]
                    )
                    nc.vector.tensor_mul(
                        ctxT_sb[64:128, tp, ms], raw_o[64:128, rs], rt[64:128, rs]
                    )
                    # ps_st banks are free in the tail (no more scores)
                    for _ in range(2):
                        emit_outproj_tile(
                            *outproj_q.pop(0),
                            pool=(ps_misc if npop % 2 else ps_st),
                            tag=("misc" if npop % 2 else "sT"),
                        )
                        npop += 1

    nc.compile()
    return nc


_NC = None
LAST_RESULTS = None


def _get_nc():
    global _NC
    if _NC is None:
        _NC = build_kernel()
    return _NC


def kernel(x, Wq, bq, Wk, bk, Wv, bv, Wo, bo):
    global LAST_RESULTS
    x = np.asarray(x, dtype=np.float32)
    Wq = np.asarray(Wq, dtype=np.float32)
    Wk = np.asarray(Wk, dtype=np.float32)
    Wv = np.asarray(Wv, dtype=np.float32)
    Wo = np.asarray(Wo, dtype=np.float32)
    bq = np.asarray(bq, dtype=np.float32)
    bk = np.asarray(bk, dtype=np.float32)
    bv = np.asarray(bv, dtype=np.float32)
    bo = np.asarray(bo, dtype=np.float32)

    nc = _get_nc()

    xTb = [_to_proj(x[b].T) for b in range(B)]
    in_maps = []
    for c in range(N_CORES):
        b, tp = divmod(c, TP)
        sl = slice(tp * D_LOCAL, (tp + 1) * D_LOCAL)
        in_maps.append(
            {
                "xT": xTb[b],
                "wq": _to_proj(Wq[:, sl]),
                "wk": _to_proj(Wk[:, sl]),
                "wv": _to_proj(Wv[:, sl]),
                "wo": _to_proj(Wo[sl, :]),
                "bq": np.ascontiguousarray(bq[sl]),
                "bk": np.ascontiguousarray(bk[sl]),
            }
        )

    res = run_bass_kernel_spmd(nc, in_maps, core_ids=list(range(N_CORES)))
    LAST_RESULTS = res

    # v-bias and out-bias are post-softmax linear corrections: fold here.
    bias_row = (bv @ Wo + bo).astype(np.float32)  # [D_MODEL]

    out = np.empty((B, L, D_MODEL), dtype=np.float32)
    for b in range(B):
        acc = res.results[b * TP]["out_p"].astype(np.float32)
        for tp in range(1, TP):
            acc = acc + res.results[b * TP + tp]["out_p"].astype(np.float32)
        out[b] = acc + bias_row
    return out


# revision 37
# speedup vs baseline: 1.0415x; 1.0415x over previous
"""Trainium2 Bass kernel for explicit multi-head attention.

Problem: x[2, 2048, 1024], Wq/Wk/Wv/Wo[1024, 1024] (+biases), NHEAD=16.
Sharding over 8 NeuronCores: data-parallel on batch (cores 0-3 -> b=0,
cores 4-7 -> b=1), tensor-parallel over heads (4 heads per core).  Each
core computes its 4 heads' attention plus the partial out-projection
(ctx_local @ Wo[rows_local]); partials are summed on the host, which is
mathematically the all-reduce the sharding hint asks for.  The v-bias and
out-bias are linear post-softmax corrections (sum of softmax weights is
1), so bv@Wo + bo is folded in on the host and never touches the device.

Device-side layout notes:
 - Everything is computed transposed: xT [D, L] streams through the PE as
   the moving operand, producing qT/kT [d_local, L] and v [L, d_local].
 - Scores are computed transposed per (head, Lk-tile, Lq-chunk):
   sT = kT_tile^T-contraction -> [Lk=128, Lq=512]; the even/odd head
   matmuls occupy PE row-halves 0-63/64-127 and co-execute.  Softmax runs
   without max-subtraction (score range ~[-4, 4], exp is safe in fp32).
 - The softmax denominator is produced by the PV matmul itself: the
   stationary v tile carries an extra all-ones column, so one PSUM row
   accumulates sum_k exp(s).  Even head lands ctx in psum partitions
   0-63 / denom at 64; odd head (stationary window shifted) lands ctx at
   64-127 / denom at 32, so ctxT packs two heads per 128-partition tile
   with no cross-partition moves.
 - Input DMA is column-chunk-major: xT streams per (k-tile, Lq-chunk)
   piece across three HWDGE queues (sync / vector / scalar), ordered so
   chunk 0 plus wq/wk/wv land ~12-15us in; the j-loop starts ~17us
   instead of waiting for the whole k-tile-major stream (~40us).  The
   whole projection drip schedule (k(c1..3), q(c1) and all pair-1
   groups) rides inside the attention j-loops.
 - PSUM budget (8 banks): scores double-buffer 2x[128,1024] (4), ctx
   accumulators 2x[128,512] (reuse one chunk apart, released by the
   prompt raw-copies), one "bcs" bank (denominator broadcast target at
   boundaries; chunk-0 qk drips and steady-state outproj tiles reuse it
   off-cycle), one "misc" bank (v-projection / steady drips).
 - Normalization off the critical path: DVE copies evacuate raw
   ctx+denoms at the PV stop, reciprocal_approx_fast (51-ULP, ~5x
   faster than exact) + rank-1 PE broadcasts + one DVE multiply per
   head produce normalized ctxT (bf16) while the next chunk runs.
 - The out-projection runs fully in bf16 (ctxT and Wo): single-pass PE
   matmuls and fast weight loads, vs 2-cycle/col fp32.
"""

import os
import sys

import numpy as np

for _p in ("/opt/trn_rl_repo", "/root/.axon_site/_ro/trn_rl_repo"):
    if os.path.isdir(_p) and _p not in sys.path:
        sys.path.append(_p)

import concourse.bass as bass
import concourse.mybir as mybir
import concourse.tile as tile
from concourse import bacc
from concourse.bass_utils import run_bass_kernel_spmd

# ---------------------------------------------------------------------------
# ACT table-set pinning: the only activation is Exp (plus Identity/Copy for
# helpers).  The stock chooser can alternate table sets between activation
# variants, costing an ACT_TABLE_LOAD (~1.3-2.6us) per switch; restrict the
# map so exactly one set serves everything and one load is emitted.
import concourse.hw_specs as _hw_specs

_orig_get_activation_tables = _hw_specs.get_activation_tables


def _pinned_activation_tables(module_arch):
    t = _orig_get_activation_tables(module_arch)
    pin = {
        mybir.ActivationFunctionType.Exp,
        mybir.ActivationFunctionType.Ln,
        mybir.ActivationFunctionType.Identity,
        mybir.ActivationFunctionType.Copy,
    }
    out = {}
    for name, fns in t.items():
        if name == "natural_log_exp_and_others":
            out[name] = set(fns)
        else:
            out[name] = set(fns) - pin
    return out


bacc.get_activation_tables = _pinned_activation_tables

B = 2
L = 2048
D_MODEL = 1024
NHEAD = 16
D_HEAD = 64
SCALE = 1.0 / float(np.sqrt(D_HEAD))
N_CORES = 8
TP = 4                      # tensor-parallel group size (heads split)
HEADS_PER_CORE = NHEAD // TP          # 4
D_LOCAL = HEADS_PER_CORE * D_HEAD     # 256
N_PAIRS = HEADS_PER_CORE // 2         # 2 head-pairs per core
KT = D_MODEL // 128                   # 8 contraction tiles for projections
LT = L // 128                         # 16 L tiles
NCH = L // 512                        # 4 Lq chunks of 512

F32 = mybir.dt.float32
BF16 = mybir.dt.bfloat16
ATT_DT = BF16   # attention operands (qT/kT/v_aug/exp-probs/ctxT/wo)
PROJ_DT = BF16  # projection inputs (x, Wq/Wk/Wv/Wo): halves the HBM load
OUT_DT = BF16   # out_p store dtype

# v_aug stationary layout (one tile per (pair, Lk-tile), [128, 193]):
#   cols 0:64    = v(even head)          -> even window cols 0:128
#   col  64      = ones (even denom -> even psum row 64)
#   col  65      = ones (odd denom; odd window col index 0 -> odd psum row 0,
#                  so the whole ctx_o bank evacuates with one base-0 copy)
#   cols 66:129  = zeros
#   cols 129:193 = v(odd head)           -> odd window cols 65:193
VAUG_W = 193


def _to_proj(x):
    import ml_dtypes

    return np.ascontiguousarray(np.asarray(x, np.float32).astype(ml_dtypes.bfloat16))


def build_kernel():
    nc = bacc.Bacc("TRN2", target_bir_lowering=False, debug=False)

    xT = nc.dram_tensor("xT", [D_MODEL, L], PROJ_DT, kind="ExternalInput").ap()
    wq = nc.dram_tensor("wq", [D_MODEL, D_LOCAL], PROJ_DT, kind="ExternalInput").ap()
    wk = nc.dram_tensor("wk", [D_MODEL, D_LOCAL], PROJ_DT, kind="ExternalInput").ap()
    wv = nc.dram_tensor("wv", [D_MODEL, D_LOCAL], PROJ_DT, kind="ExternalInput").ap()
    wo = nc.dram_tensor("wo", [D_LOCAL, D_MODEL], PROJ_DT, kind="ExternalInput").ap()
    bq = nc.dram_tensor("bq", [D_LOCAL], F32, kind="ExternalInput").ap()
    bk = nc.dram_tensor("bk", [D_LOCAL], F32, kind="ExternalInput").ap()
    out_p = nc.dram_tensor("out_p", [L, D_MODEL], OUT_DT, kind="ExternalOutput").ap()

    Exp = mybir.ActivationFunctionType.Exp

    with tile.TileContext(nc) as tc:
        with (
            tc.tile_pool(name="persist", bufs=1) as persist,
            tc.tile_pool(name="exp_pool", bufs=3) as exp_pool,
            tc.tile_pool(name="raw_pool", bufs=4) as raw_pool,
            tc.tile_pool(name="recip_pool", bufs=2) as recip_pool,
            tc.tile_pool(name="out_pool", bufs=4) as out_pool,
            tc.tile_pool(name="ps_st", bufs=2, space="PSUM") as ps_st,
            tc.tile_pool(name="ps_ctx", bufs=2, space="PSUM") as ps_ctx,
            tc.tile_pool(name="ps_bcs", bufs=1, space="PSUM") as ps_bcs,
            tc.tile_pool(name="ps_misc", bufs=1, space="PSUM") as ps_misc,
        ):
            # ---- input DMAs, column-chunk-major across three HWDGE queues.
            # Gates: xT(c0) + wq by ~14us (q/k projections), wv by ~15
            # (v tiles), later chunks stream well ahead of the j-loop's
            # consumption.  Piece = [128, 512] per (k-tile, chunk).
            xT_sb = persist.tile([128, KT, L], PROJ_DT)
            wq_sb = persist.tile([128, KT, D_LOCAL], PROJ_DT)
            wk_sb = persist.tile([128, KT, D_LOCAL], PROJ_DT)
            wv_sb = persist.tile([128, KT, D_LOCAL], PROJ_DT)
            wo_sb = persist.tile([128, N_PAIRS, D_MODEL], PROJ_DT)
            bq_sb = persist.tile([128, D_LOCAL // 128], F32)
            bk_sb = persist.tile([128, D_LOCAL // 128], F32)

            def xt_piece(eng, k, c):
                eng.dma_start(
                    xT_sb[:, k, c * 512 : (c + 1) * 512],
                    xT[k * 128 : (k + 1) * 128, c * 512 : (c + 1) * 512],
                )

            # sync queue: wk, xT-even chunk 0, wv, then xT-even chunks 1-3
            nc.sync.dma_start(wk_sb[:], wk.rearrange("(k p) n -> p k n", p=128))
            for k in range(0, KT, 2):
                xt_piece(nc.sync, k, 0)
            nc.sync.dma_start(wv_sb[:], wv.rearrange("(k p) n -> p k n", p=128))
            for c in range(1, NCH):
                for k in range(0, KT, 2):
                    xt_piece(nc.sync, k, c)
            # scalar queue: biases, xT-odd chunk 0, wq, xT-odd chunks 1-3, wo
            nc.scalar.dma_start(bq_sb[:], bq.rearrange("(m p) -> p m", p=128))
            nc.scalar.dma_start(bk_sb[:], bk.rearrange("(m p) -> p m", p=128))
            for k in range(1, KT, 2):
                xt_piece(nc.scalar, k, 0)
            nc.scalar.dma_start(wq_sb[:], wq.rearrange("(k p) n -> p k n", p=128))
            for c in range(1, NCH):
                for k in range(1, KT, 2):
                    xt_piece(nc.scalar, k, c)
            nc.scalar.dma_start(wo_sb[:], wo.rearrange("(k p) n -> p k n", p=128))

            ones128 = persist.tile([128, 512], F32)
            nc.vector.memset(ones128[:], 1.0)

            qT_sb = persist.tile([128, N_PAIRS, L], ATT_DT)
            kT_sb = persist.tile([128, N_PAIRS, L], ATT_DT)
            vaug = persist.tile([128, N_PAIRS, LT, VAUG_W], ATT_DT)
            ctxT_sb = persist.tile([128, N_PAIRS, L], ATT_DT)

            nc.vector.memset(vaug[:, :, :, 64:129], 0.0)
            nc.vector.memset(vaug[:, :, :, 64:66], 1.0)

            # ---- v projection, one Lk tile (no bias: folded on host) ----
            def emit_v(lt, pool=ps_misc, tag="misc"):
                ps = pool.tile([128, D_LOCAL], F32, tag=tag)
                for k in range(KT):
                    nc.tensor.matmul(
                        ps[:],
                        xT_sb[:, k, lt * 128 : (lt + 1) * 128],
                        wv_sb[:, k, :],
                        start=(k == 0),
                        stop=(k == KT - 1),
                    )
                for p in range(N_PAIRS):
                    nc.vector.tensor_copy(
                        vaug[:, p, lt, 0:64], ps[:, p * 128 : p * 128 + 64]
                    )
                    nc.vector.tensor_copy(
                        vaug[:, p, lt, 129:193],
                        ps[:, p * 128 + 64 : p * 128 + 128],
                    )

            # ---- one qT/kT projection group: tensor t (0=q, 1=k), pair m,
            # Lq chunk c ----
            open_qk = {}

            def emit_qk(t, m, c, pool, tag, half=None):
                # half=0 emits k 0..3 (opens the PSUM group), half=1 emits
                # k 4..7 + the bias evacuation; half=None does both.
                w_sb, b_sb, dst = ((wq_sb, bq_sb, qT_sb), (wk_sb, bk_sb, kT_sb))[t]
                if half == 1:
                    ps = open_qk.pop((t, m, c))
                else:
                    ps = pool.tile([128, 512], F32, tag=tag)
                ks = {None: range(KT), 0: range(KT // 2), 1: range(KT // 2, KT)}[half]
                for k in ks:
                    nc.tensor.matmul(
                        ps[:],
                        w_sb[:, k, m * 128 : (m + 1) * 128],
                        xT_sb[:, k, c * 512 : (c + 1) * 512],
                        start=(k == 0),
                        stop=(k == KT - 1),
                    )
                if half == 0:
                    open_qk[(t, m, c)] = ps
                    return
                nc.vector.tensor_scalar_add(
                    dst[:, m, c * 512 : (c + 1) * 512], ps[:], b_sb[:, m : m + 1]
                )

            def emit_norm_copies(p, c, ctx_e, ctx_o):
                # Evacuate raw ctx+denoms: one base-0 copy per bank (denom_e
                # rides at row 64 of ctx_e, denom_o at row 0 of ctx_o), which
                # releases both ctx PSUM banks as fast as possible.
                raw_e = raw_pool.tile([128, 512], F32, tag="raw")
                raw_o = raw_pool.tile([128, 512], F32, tag="raw")
                nc.vector.tensor_copy(raw_e[0:65, :], ctx_e[0:65, :])
                nc.vector.tensor_copy(raw_o[:, :], ctx_o[:, :])
                return {"p": p, "c": c, "raw_e": raw_e, "raw_o": raw_o}

            def emit_norm_bcast(st, half):
                # broadcast one denom row across partitions with a rank-1 PE
                # matmul (a broadcast DMA costs ~6.4us issue-to-semaphore)
                if half == 0:
                    st["bcs"] = ps_bcs.tile(
                        [128, 512], F32, tag="bcs", name="bcs"
                    )
                    nc.tensor.matmul(
                        st["bcs"][0:64, :], ones128[64:65, 0:64],
                        st["raw_e"][64:65, :], start=True, stop=True,
                    )
                else:
                    nc.tensor.matmul(
                        st["bcs"][64:128, :], ones128[0:1, 0:64],
                        st["raw_o"][0:1, :], start=True, stop=True,
                    )

            def emit_norm_mul(st):
                # reciprocal_approx_fast (51-ULP, full-partition span) + the
                # normalize multiplies, producing bf16 ctxT
                p, c, bcs = st["p"], st["c"], st["bcs"]
                sl = slice(c * 512, (c + 1) * 512)
                rt = recip_pool.tile([128, 512], F32, tag="rt", name="rt")
                nc.vector.reciprocal_approx_fast(rt[:, :], bcs[:, :])
                nc.vector.tensor_mul(
                    ctxT_sb[0:64, p, sl], st["raw_e"][0:64, :], rt[0:64, :]
                )
                nc.vector.tensor_mul(
                    ctxT_sb[64:128, p, sl], st["raw_o"][64:128, :], rt[64:128, :]
                )

            def emit_outproj_tile(c, idx, pool=ps_bcs, tag="bcs"):
                # one [128, 512] tile of out[L, D] for Lq chunk c (no bias:
                # folded on host); bf16 output halves the store DMA.
                m = 4 * c + idx // 2
                n = idx % 2
                po = pool.tile([128, 512], F32, tag=tag)
                for k in range(N_PAIRS):
                    nc.tensor.matmul(
                        po[:],
                        ctxT_sb[:, k, m * 128 : (m + 1) * 128],
                        wo_sb[:, k, n * 512 : (n + 1) * 512],
                        start=(k == 0),
                        stop=(k == N_PAIRS - 1),
                    )
                ot = out_pool.tile([128, 512], OUT_DT, tag="ot")
                nc.vector.tensor_copy(ot[:], po[:])
                nc.sync.dma_start(
                    out_p[m * 128 : (m + 1) * 128, n * 512 : (n + 1) * 512], ot[:]
                )

            # ---- emission schedule ----
            # PE warm-up: dense dummy matmuls on resident data while the
            # input DMAs land, so the HAM clock-gate ramps before the real
            # projections start.
            # No PE warm-up: the cold-clock window is ~3.4us of activity
            # (absorbed by the first projection halves), while dummy matmuls
            # burn HAM power budget that gets repaid later in 4/8-throttle
            # windows.
            # head: k(pair0, c0) then q(pair0, c0) then the first two v
            # tiles -- the minimum for attention chunk 0's j-loop to start.
            # Half-granularity emission lets each start on partial xT/weights.
            emit_qk(1, 0, 0, ps_ctx, "ctx", half=0)
            emit_qk(1, 0, 0, ps_ctx, "ctx", half=1)
            emit_qk(0, 0, 0, ps_ctx, "ctx", half=0)
            emit_qk(0, 0, 0, ps_ctx, "ctx", half=1)
            emit_v(0, ps_ctx, "ctx")
            emit_v(1, ps_ctx, "ctx")

            def emit_pv(pv):
                # even head: M=65 stationary (ctx 0:64 + denom row 64) -- the
                # zero columns 65:128 would only burn PE array power.  The
                # odd head's ones column must stay 64 window-columns from its
                # v block (denom -> row 0, ctx -> rows 64:128), so it keeps
                # the full 128-wide window.
                pv_e, pv_o, pv_p, pv_j, pv_et = pv
                nc.tensor.matmul(
                    pv_e[0:65, :], vaug[:, pv_p, pv_j, 0:65], pv_et[:, 0:512],
                    start=(pv_j == 0), stop=(pv_j == LT - 1),
                )
                nc.tensor.matmul(
                    pv_o[:], vaug[:, pv_p, pv_j, 65:193], pv_et[:, 512:1024],
                    start=(pv_j == 0), stop=(pv_j == LT - 1),
                )

            # chunk 0 drips (bcs slot, free until the first boundary):
            # j -> (group, half); kT(c') needed by j=4c'.
            chunk0_drips = {
                1: ((1, 0, 1), 0), 2: ((1, 0, 1), 1),
                4: ((1, 0, 2), 0), 5: ((1, 0, 2), 1),
                8: ((1, 0, 3), 0), 9: ((1, 0, 3), 1),
                11: ((0, 0, 1), 0), 12: ((0, 0, 1), 1),
            }
            # steady-state drips (misc slot, odd j >= 5 to keep the chunk
            # boundary region clear), chunk_i -> groups.  Chunks 5-7 carry
            # the out-projection stream and stay drip-free; chunks 3-4
            # (exp-paced, PE slack) absorb the extra pair-1 q groups.
            qk_drip = {
                1: [(1, 1, 0), (0, 0, 2)],
                2: [(1, 1, 1), (0, 0, 3)],
                3: [(1, 1, 2), (0, 1, 0), (0, 1, 3)],
                4: [(1, 1, 3), (0, 1, 1), (0, 1, 2)],
            }

            outproj_q = []
            pending = None        # chunk awaiting normalize (flush + copies)
            norm = None           # normalize stage state across j=2..4
            pending_pv = None     # software-pipelined PV emission (lags 1 j)
            for p in range(N_PAIRS):
                for c in range(NCH):
                    chunk_i = p * NCH + c
                    drips = [
                        (g, h) for g in qk_drip.get(chunk_i, []) for h in (0, 1)
                    ]
                    drip_js = iter((5, 7, 9, 11, 13, 15))
                    next_drip_j = next(drip_js) if drips else None
                    ctx_e = ps_ctx.tile([128, 512], F32, tag="ctx")
                    ctx_o = ps_ctx.tile([128, 512], F32, tag="ctx")
                    for j in range(LT):
                        sT = ps_st.tile([128, 1024], F32, tag="sT")
                        nc.tensor.matmul(
                            sT[:, 0:512],
                            kT_sb[0:64, p, j * 128 : (j + 1) * 128],
                            qT_sb[0:64, p, c * 512 : (c + 1) * 512],
                            start=True,
                            stop=True,
                        )
                        nc.tensor.matmul(
                            sT[:, 512:1024],
                            kT_sb[64:128, p, j * 128 : (j + 1) * 128],
                            qT_sb[64:128, p, c * 512 : (c + 1) * 512],
                            start=True,
                            stop=True,
                        )
                        et = exp_pool.tile([128, 1024], ATT_DT, tag="et")
                        nc.scalar.activation(et[:], sT[:], Exp, scale=SCALE)
                        if j == 0 and pending is not None:
                            # cover the last exp's latency with a filler,
                            # then flush the previous chunk's last PV
                            # (unlagged) and evacuate it -- the ACT engine
                            # streams exps seamlessly across the boundary.
                            if outproj_q:
                                emit_outproj_tile(*outproj_q.pop(0))
                            emit_pv(pending_pv)
                            pending_pv = None
                            prev_p, prev_c = pending[0], pending[1]
                            norm = emit_norm_copies(*pending)
                            pending = None
                            if prev_p == N_PAIRS - 1:
                                outproj_q.extend(
                                    (prev_c, idx) for idx in range(8)
                                )
                        elif pending_pv is not None:
                            emit_pv(pending_pv)
                        pending_pv = (ctx_e, ctx_o, p, j, et)
                        if norm is not None:
                            if j == 2:
                                emit_norm_bcast(norm, 0)
                            elif j == 3:
                                emit_norm_bcast(norm, 1)
                            elif j == 4:
                                emit_norm_mul(norm)
                                norm = None
                        if chunk_i == 0:
                            if j <= 13:
                                emit_v(j + 2)
                            if j in chunk0_drips:
                                g, h = chunk0_drips[j]
                                emit_qk(*g, ps_bcs, "bcs", half=h)
                        elif j == next_drip_j and drips:
                            g, h = drips.pop(0)
                            emit_qk(*g, ps_misc, "misc", half=h)
                            next_drip_j = next(drip_js, None)
                        if (
                            j >= 4
                            and outproj_q
                            and (j % 2 == 0 or len(outproj_q) > 4)
                        ):
                            emit_outproj_tile(*outproj_q.pop(0))
                    pending = (p, c, ctx_e, ctx_o)
            # drain the pipelined last PV, then the tail
            emit_pv(pending_pv)
            # tail: pipeline the last chunk's normalize with its
            # out-projection -- per-m-tile multiplies release each
            # out-projection tile as early as possible.
            st = emit_norm_copies(*pending)
            tp, tc_ = st["p"], st["c"]
            emit_norm_bcast(st, 0)
            emit_norm_bcast(st, 1)
            raw_e, raw_o, bcs = st["raw_e"], st["raw_o"], st["bcs"]
            outproj_q.extend((NCH - 1, idx) for idx in range(8))
            rt = recip_pool.tile([128, 512], F32, tag="rt")
            npop = 0
            for half in range(2):
                hs = slice(half * 256, (half + 1) * 256)
                nc.vector.reciprocal_approx_fast(rt[:, hs], bcs[:, hs])
                for ml in (2 * half, 2 * half + 1):
                    ms = slice(tc_ * 512 + ml * 128, tc_ * 512 + (ml + 1) * 128)
                    rs = slice(ml * 128, (ml + 1) * 128)
                    nc.vector.tensor_mul(
                        ctxT_sb[0:64, tp, ms], raw_e[0:64, rs], rt[0:64, rs]
                    )
                    nc.vector.tensor_mul(
                        ctxT_sb[64:128, tp, ms], raw_o[64:128, rs], rt[64:128, rs]
                    )
                    # ps_st banks are free in the tail (no more scores)
                    for _ in range(2):
                        emit_outproj_tile(
                            *outproj_q.pop(0),
                            pool=(ps_misc if npop % 2 else ps_st),
                            tag=("misc" if npop % 2 else "sT"),
                        )
                        npop += 1

    nc.compile()
    return nc


_NC = None
LAST_RESULTS = None


def _get_nc():
    global _NC
    if _NC is None:
        _NC = build_kernel()
    return _NC


def kernel(x, Wq, bq, Wk, bk, Wv, bv, Wo, bo):
    global LAST_RESULTS
    x = np.asarray(x, dtype=np.float32)
    Wq = np.asarray(Wq, dtype=np.float32)
    Wk = np.asarray(Wk, dtype=np.float32)
    Wv = np.asarray(Wv, dtype=np.float32)
    Wo = np.asarray(Wo, dtype=np.float32)
    bq = np.asarray(bq, dtype=np.float32)
    bk = np.asarray(bk, dtype=np.float32)
    bv = np.asarray(bv, dtype=np.float32)
    bo = np.asarray(bo, dtype=np.float32)

    nc = _get_nc()

    xTb = [_to_proj(x[b].T) for b in range(B)]
    in_maps = []
    for c in range(N_CORES):
        b, tp = divmod(c, TP)
        sl = slice(tp * D_LOCAL, (tp + 1) * D_LOCAL)
        in_maps.append(
            {
                "xT": xTb[b],
                "wq": _to_proj(Wq[:, sl]),
                "wk": _to_proj(Wk[:, sl]),
                "wv": _to_proj(Wv[:, sl]),
                "wo": _to_proj(Wo[sl, :]),
                "bq": np.ascontiguousarray(bq[sl]),
                "bk": np.ascontiguousarray(bk[sl]),
            }
        )

    res = run_bass_kernel_spmd(nc, in_maps, core_ids=list(range(N_CORES)))
    LAST_RESULTS = res

    # v-bias and out-bias are post-softmax linear corrections: fold here.
    bias_row = (bv @ Wo + bo).astype(np.float32)  # [D_MODEL]

    out = np.empty((B, L, D_MODEL), dtype=np.float32)
    for b in range(B):
        acc = res.results[b * TP]["out_p"].astype(np.float32)
        for tp in range(1, TP):
            acc = acc + res.results[b * TP + tp]["out_p"].astype(np.float32)
        out[b] = acc + bias_row
    return out
